# revision 2
# baseline (speedup 1.0000x reference)
"""Transformer block (B=16, N=1024, D=768, H=12, MLP=3072) on 8 trn2 cores.

Strategy: data-parallel over batch (2 batch items = 2048 tokens per core).
All matmuls in fp16 on the PE (fp32 psum accumulate), weight-stationary
d-major layout; LN gains/biases folded into the following weight matrices
on the host; attention computed per head-pair with scores^T kt-major so
softmax denominators ride the ctx matmul as an appended ones-column of V.

Self-contained: hardcodes all shapes; no repo file reads.
"""

import numpy as np

import concourse.bass as bass
import concourse.bacc as bacc
import concourse.mybir as mybir
import concourse.tile as tile
from concourse import bass_utils

f32 = mybir.dt.float32
f16 = mybir.dt.float16
AF = mybir.ActivationFunctionType
Alu = mybir.AluOpType
ts = bass.ts

B, N, D, H, HD, MLP = 16, 1024, 768, 12, 64, 3072
NC = 8
BPC = B // NC            # batch items per core
T = BPC * N              # tokens per core = 2048
NT = T // 128            # token tiles = 16
NTB = T // 512           # token blocks = 4
DT = D // 128            # 6 din tiles
QKVT = 3 * DT            # 18 qkv dout tiles
MT = MLP // 128          # 24 hidden tiles
NPAIR = H // 2           # 6 head pairs
NKT = N // 128           # 8 key tiles per sequence
NQH = N // 512           # 2 query halves per sequence
SCALE = 0.125
EPS = 1e-6


def build_nc():
    nc = bacc.Bacc("TRN2", target_bir_lowering=False)

    x_d = nc.dram_tensor("x", [T, D], f32, kind="ExternalInput")
    wqkv_d = nc.dram_tensor("wqkv", [D, 3 * D], f16, kind="ExternalInput")
    bqkv_d = nc.dram_tensor("bqkv", [3 * D], f32, kind="ExternalInput")
    wo_d = nc.dram_tensor("wo", [D, D], f16, kind="ExternalInput")
    bo_d = nc.dram_tensor("bo", [D], f32, kind="ExternalInput")
    w1_d = nc.dram_tensor("w1", [D, MLP], f16, kind="ExternalInput")
    b1_d = nc.dram_tensor("b1", [MLP], f32, kind="ExternalInput")
    w2_d = nc.dram_tensor("w2", [MLP, D], f16, kind="ExternalInput")
    b2_d = nc.dram_tensor("b2", [D], f32, kind="ExternalInput")
    out_d = nc.dram_tensor("out", [T, D], f32, kind="ExternalOutput")

    ctx_d = nc.dram_tensor("ctx_spill", [DT, 128, T], f16)
    x2_d = nc.dram_tensor("x2_spill", [T, D], f32)

    with tile.TileContext(nc) as tc:
        with (
            tc.tile_pool(name="outer", bufs=1) as outer,
            tc.tile_pool(name="psum", bufs=1, space="PSUM") as psp,
        ):
            # biases + eps live for the whole kernel
            bqkv_sb = outer.tile([128, QKVT], f32, tag="bqkv")
            nc.sync.dma_start(bqkv_sb, bqkv_d[:].rearrange("(t p) -> p t", p=128))
            bo_sb = outer.tile([128, DT], f32, tag="bo")
            nc.sync.dma_start(bo_sb, bo_d[:].rearrange("(t p) -> p t", p=128))
            b1_sb = outer.tile([128, MT], f32, tag="b1")
            nc.sync.dma_start(b1_sb, b1_d[:].rearrange("(t p) -> p t", p=128))
            b2_sb = outer.tile([128, DT], f32, tag="b2")
            nc.sync.dma_start(b2_sb, b2_d[:].rearrange("(t p) -> p t", p=128))
            eps_sb = outer.tile([128, 1], f32, tag="eps")
            nc.vector.memset(eps_sb, EPS)

            h2T = [
                outer.tile([128, T], f16, tag=f"h2T{j}", name=f"h2T{j}")
                for j in range(DT)
            ]

            def layernorm_tile(pool, x_tile, out_f16, tagpfx):
                """out_f16 = (x - mean)/sqrt(var+eps), token-major [128, D]."""
                xg = x_tile.rearrange("p (s c) -> p s c", c=256)
                stats = pool.tile([128, 3, 6], f32, tag=f"{tagpfx}_stats",
                                  name=f"{tagpfx}_stats", bufs=2)
                for s in range(3):
                    nc.vector.bn_stats(stats[:, s, :], xg[:, s, :])
                mv = pool.tile([128, 2], f32, tag=f"{tagpfx}_mv",
                               name=f"{tagpfx}_mv", bufs=2)
                nc.vector.bn_aggr(mv, stats)
                rstd = pool.tile([128, 1], f32, tag=f"{tagpfx}_rstd",
                                 name=f"{tagpfx}_rstd", bufs=2)
                nc.scalar.activation(rstd, mv[:, 1:2], AF.Sqrt, bias=eps_sb)
                nc.vector.reciprocal(rstd, rstd)
                nc.vector.tensor_scalar(
                    out=out_f16, in0=x_tile, scalar1=mv[:, 0:1], scalar2=rstd,
                    op0=Alu.subtract, op1=Alu.mult,
                )

            # ───────── Phase 1-3: LN1, QKV, attention ─────────
            with tc.tile_pool(name="s1", bufs=1) as s1:
                h1T = [
                    s1.tile([128, T], f16, tag=f"h1T{j}", name=f"h1T{j}")
                    for j in range(DT)
                ]
                with nc.named_scope("ln1"):
                    for t in range(NT):
                        x_t = s1.tile([128, D], f32, tag="x_t", name="x_t", bufs=3)
                        nc.sync.dma_start(x_t, x_d[ts(t, 128), :])
                        h_t = s1.tile([128, D], f16, tag="h_t", name="h_t", bufs=3)
                        layernorm_tile(s1, x_t, h_t, "ln1")
                        for j in range(DT):
                            nc.sync.dma_start(
                                h1T[j][:, ts(t, 128)], h_t[:, ts(j, 128)],
                                transpose=True,
                            )

                wqkv_sb = [
                    s1.tile([128, 3 * D], f16, tag=f"wqkv{j}", name=f"wqkv{j}")
                    for j in range(DT)
                ]
                for j in range(DT):
                    nc.sync.dma_start(wqkv_sb[j], wqkv_d[ts(j, 128), :])

                qkvT = [
                    s1.tile([128, T], f16, tag=f"qkvT{p}", name=f"qkvT{p}")
                    for p in range(QKVT)
                ]
                with nc.named_scope("qkv"):
                    for p in range(QKVT):
                        pss = [
                            psp.tile([128, 512], f32, tag=f"pj{i}",
                                     name=f"qkv_ps{p}_{i}", bufs=1)
                            for i in range(NTB)
                        ]
                        for j in range(DT):
                            for tb in range(NTB):
                                nc.tensor.matmul(
                                    pss[tb],
                                    wqkv_sb[j][:, ts(p, 128)],
                                    h1T[j][:, ts(tb, 512)],
                                    start=(j == 0), stop=(j == DT - 1),
                                )
                        for tb in range(NTB):
                            nc.scalar.activation(
                                qkvT[p][:, ts(tb, 512)], pss[tb],
                                AF.Identity, bias=bqkv_sb[:, p : p + 1],
                            )

                with nc.named_scope("attn"):
                    for p in range(NPAIR):
                        qT = qkvT[3 * p]
                        kT = qkvT[3 * p + 1]
                        vT = qkvT[3 * p + 2]
                        # v token-major with ones column appended
                        vaug = {}
                        for hh in range(2):
                            for b in range(BPC):
                                for kt in range(NKT):
                                    va = s1.tile(
                                        [128, 65], f16, tag="vaug",
                                        name=f"vaug{p}_{hh}_{b}_{kt}", bufs=36,
                                    )
                                    nc.sync.dma_start(
                                        va[:, 0:64],
                                        vT[ts(hh, 64), b * N + kt * 128:
                                           b * N + kt * 128 + 128],
                                        transpose=True,
                                    )
                                    nc.vector.memset(va[:, 64:65], 1.0)
                                    vaug[(hh, b, kt)] = va
                        for b in range(BPC):
                            for qh in range(NQH):
                                q0 = b * N + qh * 512
                                pts = {}
                                for kt in range(NKT):
                                    for hh in range(2):
                                        ps_s = psp.tile(
                                            [128, 512], f32, tag=f"scs{hh}",
                                            name=f"s_ps{p}_{b}_{qh}_{kt}_{hh}",
                                            bufs=1,
                                        )
                                        nc.tensor.matmul(
                                            ps_s,
                                            kT[ts(hh, 64),
                                               b * N + kt * 128:
                                               b * N + kt * 128 + 128],
                                            qT[ts(hh, 64), q0:q0 + 512],
                                            start=True, stop=True,
                                        )
                                        pt = s1.tile(
                                            [128, 512], f16, tag="pt",
                                            name=f"pt{p}_{b}_{qh}_{kt}_{hh}",
                                            bufs=18,
                                        )
                                        nc.scalar.activation(
                                            pt, ps_s, AF.Exp, scale=SCALE
                                        )
                                        pts[(kt, hh)] = pt
                                for hh in range(2):
                                    ps_c = psp.tile(
                                        [65, 512], f32, tag=f"ctx{hh}",
                                        name=f"c_ps{p}_{b}_{qh}_{hh}", bufs=1,
                                    )
                                    for kt in range(NKT):
                                        nc.tensor.matmul(
                                            ps_c,
                                            vaug[(hh, b, kt)],
                                            pts[(kt, hh)],
                                            start=(kt == 0),
                                            stop=(kt == NKT - 1),
                                        )
                                    recip = s1.tile(
                                        [1, 512], f32, tag="recip",
                                        name=f"recip{p}_{b}_{qh}_{hh}", bufs=4,
                                    )
                                    nc.vector.reciprocal(recip, ps_c[64:65, :])
                                    rb = s1.tile(
                                        [64, 512], f32, tag="rb",
                                        name=f"rb{p}_{b}_{qh}_{hh}", bufs=4,
                                    )
                                    nc.gpsimd.partition_broadcast(rb, recip)
                                    cev = s1.tile(
                                        [64, 512], f16, tag="cev",
                                        name=f"cev{p}_{b}_{qh}_{hh}", bufs=4,
                                    )
                                    nc.vector.tensor_mul(cev, ps_c[0:64, :], rb)
                                    nc.sync.dma_start(
                                        ctx_d[p, ts(hh, 64), q0:q0 + 512], cev
                                    )

            # ───────── Phase 4: out-projection, residual, LN2 ─────────
            with tc.tile_pool(name="s2", bufs=1) as s2:
                wo_sb = [
                    s2.tile([128, D], f16, tag=f"wo{j}", name=f"wo{j}")
                    for j in range(DT)
                ]
                for j in range(DT):
                    nc.sync.dma_start(wo_sb[j], wo_d[ts(j, 128), :])
                with nc.named_scope("oproj"):
                    for tb in range(NTB):
                        ctx_in = [
                            s2.tile([128, 512], f16, tag=f"ctxin{j}",
                                    name=f"ctxin{tb}_{j}", bufs=2)
                            for j in range(DT)
                        ]
                        for j in range(DT):
                            nc.sync.dma_start(
                                ctx_in[j], ctx_d[j, :, ts(tb, 512)]
                            )
                        attn_tok = [
                            s2.tile([128, D], f16, tag="attn_tok",
                                    name=f"attok{tb}_{i}", bufs=6)
                            for i in range(4)
                        ]
                        for d in range(DT):
                            ps = psp.tile([128, 512], f32, tag=f"pj{d % 4}",
                                          name=f"o_ps{tb}_{d}", bufs=1)
                            for j in range(DT):
                                nc.tensor.matmul(
                                    ps, wo_sb[j][:, ts(d, 128)], ctx_in[j],
                                    start=(j == 0), stop=(j == DT - 1),
                                )
                            attnT = s2.tile([128, 512], f16, tag="attnT",
                                            name=f"attnT{tb}_{d}", bufs=4)
                            nc.scalar.activation(
                                attnT, ps, AF.Identity, bias=bo_sb[:, d : d + 1]
                            )
                            for i in range(4):
                                nc.sync.dma_start(
                                    attn_tok[i][:, ts(d, 128)],
                                    attnT[:, ts(i, 128)],
                                    transpose=True,
                                )
                        for i in range(4):
                            t = tb * 4 + i
                            xr = s2.tile([128, D], f32, tag="xr",
                                         name=f"xr{t}", bufs=3)
                            nc.sync.dma_start(xr, x_d[ts(t, 128), :])
                            x2t = s2.tile([128, D], f32, tag="x2t",
                                          name=f"x2t{t}", bufs=3)
                            nc.vector.tensor_add(x2t, xr, attn_tok[i])
                            nc.sync.dma_start(x2_d[ts(t, 128), :], x2t)
                            h2_t = s2.tile([128, D], f16, tag="h2_t",
                                           name=f"h2_t{t}", bufs=3)
                            layernorm_tile(s2, x2t, h2_t, "ln2")
                            for j in range(DT):
                                nc.sync.dma_start(
                                    h2T[j][:, ts(t, 128)], h2_t[:, ts(j, 128)],
                                    transpose=True,
                                )

            # ───────── Phase 5: MLP ─────────
            with tc.tile_pool(name="s3", bufs=1) as s3:
                w1_sb = [
                    s3.tile([128, MLP], f16, tag=f"w1_{j}", name=f"w1_{j}")
                    for j in range(DT)
                ]
                for j in range(DT):
                    nc.sync.dma_start(w1_sb[j], w1_d[ts(j, 128), :])
                w2_sb = [
                    s3.tile([128, D], f16, tag=f"w2_{h}", name=f"w2_{h}")
                    for h in range(MT)
                ]
                for h in range(MT):
                    nc.sync.dma_start(w2_sb[h], w2_d[ts(h, 128), :])

                with nc.named_scope("mlp"):
                    for tb in range(NTB):
                        mts = []
                        for hh in range(MT):
                            ps = psp.tile([128, 512], f32, tag=f"pj{hh % 2}",
                                          name=f"f1_ps{tb}_{hh}", bufs=1)
                            for j in range(DT):
                                nc.tensor.matmul(
                                    ps, w1_sb[j][:, ts(hh, 128)],
                                    h2T[j][:, ts(tb, 512)],
                                    start=(j == 0), stop=(j == DT - 1),
                                )
                            mt = s3.tile([128, 512], f16, tag="mt",
                                         name=f"mt{tb}_{hh}", bufs=48)
                            nc.scalar.activation(
                                mt, ps, AF.Gelu, bias=b1_sb[:, hh : hh + 1]
                            )
                            mts.append(mt)
                        out_tok = [
                            s3.tile([128, D], f16, tag="out_tok",
                                    name=f"outtok{tb}_{i}", bufs=6)
                            for i in range(4)
                        ]
                        for d in range(DT):
                            ps2 = psp.tile([128, 512], f32, tag=f"pj{2 + d % 2}",
                                           name=f"f2_ps{tb}_{d}", bufs=1)
                            for hh in range(MT):
                                nc.tensor.matmul(
                                    ps2, w2_sb[hh][:, ts(d, 128)], mts[hh],
                                    start=(hh == 0), stop=(hh == MT - 1),
                                )
                            outT = s3.tile([128, 512], f16, tag="outT",
                                           name=f"outT{tb}_{d}", bufs=4)
                            nc.scalar.activation(
                                outT, ps2, AF.Identity, bias=b2_sb[:, d : d + 1]
                            )
                            for i in range(4):
                                nc.sync.dma_start(
                                    out_tok[i][:, ts(d, 128)],
                                    outT[:, ts(i, 128)],
                                    transpose=True,
                                )
                        for i in range(4):
                            t = tb * 4 + i
                            x2r = s3.tile([128, D], f32, tag="x2r",
                                          name=f"x2r{t}", bufs=3)
                            nc.sync.dma_start(x2r, x2_d[ts(t, 128), :])
                            outf = s3.tile([128, D], f32, tag="outf",
                                           name=f"outf{t}", bufs=3)
                            nc.vector.tensor_add(outf, x2r, out_tok[i])
                            nc.sync.dma_start(out_d[ts(t, 128), :], outf)

    nc.compile()
    return nc


def host_prep(inputs):
    """Fold LN gains/biases into the adjacent weights; cast to fp16."""
    fp = np.float32
    wq, wk, wv = (np.asarray(inputs[k], fp) for k in ("wq", "wk", "wv"))
    bq, bk, bv = (np.asarray(inputs[k], fp) for k in ("bq", "bk", "bv"))
    g1 = np.asarray(inputs["ln1_g"], fp)
    b1l = np.asarray(inputs["ln1_b"], fp)
    g2 = np.asarray(inputs["ln2_g"], fp)
    b2l = np.asarray(inputs["ln2_b"], fp)
    w1 = np.asarray(inputs["w1"], fp)
    b1 = np.asarray(inputs["b1"], fp)

    wq_e, bq_e = g1[:, None] * wq, bq + b1l @ wq
    wk_e, bk_e = g1[:, None] * wk, bk + b1l @ wk
    wv_e, bv_e = g1[:, None] * wv, bv + b1l @ wv
    w1_e, b1_e = g2[:, None] * w1, b1 + b2l @ w1

    wqkv = np.empty((D, 3 * D), fp)
    bqkv = np.empty(3 * D, fp)
    for p in range(NPAIR):
        c = 128 * p
        for s, (w_e, b_e) in enumerate(((wq_e, bq_e), (wk_e, bk_e), (wv_e, bv_e))):
            dst = 384 * p + 128 * s
            wqkv[:, dst:dst + 128] = w_e[:, c:c + 128]
            bqkv[dst:dst + 128] = b_e[c:c + 128]

    return {
        "wqkv": wqkv.astype(np.float16),
        "bqkv": bqkv,
        "wo": np.asarray(inputs["wo"], fp).astype(np.float16),
        "bo": np.asarray(inputs["bo"], fp),
        "w1": w1_e.astype(np.float16),
        "b1": b1_e,
        "w2": np.asarray(inputs["w2"], fp).astype(np.float16),
        "b2": np.asarray(inputs["b2"], fp),
    }


_NC_CACHE = []


def _get_nc():
    if not _NC_CACHE:
        _NC_CACHE.append(build_nc())
    return _NC_CACHE[0]


def run(inputs, trace=False):
    nc = _get_nc()
    shared = host_prep(inputs)
    x = np.asarray(inputs["x"], np.float32)
    in_maps = [
        {**shared, "x": np.ascontiguousarray(x[c * BPC:(c + 1) * BPC].reshape(T, D))}
        for c in range(NC)
    ]
    res = bass_utils.run_bass_kernel_spmd(
        nc, in_maps, core_ids=list(range(NC)), trace=trace
    )
    out = np.concatenate(
        [res.results[c]["out"].reshape(BPC, N, D) for c in range(NC)], axis=0
    )
    return out, res


def kernel(**inputs):
    out, _ = run(inputs, trace=False)
    return out
